# revision 1
# baseline (speedup 1.0000x reference)
"""DGCNN forward on 8 TRN2 NeuronCores — data-parallel over batch.

kernel(**inputs) takes the full inputs from setup_inputs() and returns the
full [8, 40] output. Internally: one sample per core; per core the whole
network runs on-chip.

Math restructure (exact, exploits monotone LeakyReLU + positive BN scale):
  EdgeConv(x)[n,o] = lrelu( max_k proj[idx[n,k],o] + a[n,o] )
    proj = x @ (wd*s).T            (wd = w[:, :C], s = bn scale)
    a    = x @ ((wc-wd)*s).T + t   (wc = w[:, C:], t = bn shift)
  pairwise -||xi-xj||^2 via one PE matmul of augmented matrices:
    pd = [2xT; 1; d2]^T . [xT; -d2; -1]
  top-20 via DVE max8/max_index/match_replace (3 rounds; duplicate-value
  semantics verified on HW to match lax.top_k tie-breaking), gather of proj
  rows via per-k indirect DMA, k-max via one strided tensor_reduce.
"""
import numpy as np
from contextlib import ExitStack

import concourse.bass as bass
import concourse.mybir as mybir
import concourse.tile as tile
from concourse import bacc
from concourse.bass import IndirectOffsetOnAxis
from concourse.bass_utils import run_bass_kernel_spmd
from concourse.masks import make_identity

P = 128
N = 2048
NBLK = N // P
K = 20
EPS = 1e-5
NEG = -1e30
f32 = mybir.dt.float32
u32 = mybir.dt.uint32
AF = mybir.ActivationFunctionType
ALU = mybir.AluOpType

# (C_in, O) per EdgeConv layer
LAYERS = [(3, 64), (64, 64), (64, 128), (128, 256)]


# ---------------------------------------------------------------- program ---
def build_program():
    nc = bacc.Bacc("TRN2", target_bir_lowering=False, debug=False)

    xT_d = nc.dram_tensor("xT", [3, N], f32, kind="ExternalInput")
    pw_d, aw_d, projD = [], [], []
    for li, (C, O) in enumerate(LAYERS):
        pw_d.append(nc.dram_tensor(f"pw{li}", [C, O], f32, kind="ExternalInput"))
        aw_d.append((nc.dram_tensor(f"aw{li}", [C, O], f32, kind="ExternalInput"),
                     nc.dram_tensor(f"awt{li}", [1, O], f32, kind="ExternalInput")))
        projD.append(nc.dram_tensor(f"projD{li}", [N, O], f32))
    w5_d = [nc.dram_tensor(f"w5c{c}", [64 if c < 2 else 128, 1024], f32,
                           kind="ExternalInput") for c in range(5)]
    t5_d = nc.dram_tensor("t5", [1, 1024], f32, kind="ExternalInput")
    wf1_d = nc.dram_tensor("wf1", [2048, 512], f32, kind="ExternalInput")
    t6_d = nc.dram_tensor("t6", [1, 512], f32, kind="ExternalInput")
    wf2_d = nc.dram_tensor("wf2", [512, 256], f32, kind="ExternalInput")
    t7_d = nc.dram_tensor("t7", [1, 256], f32, kind="ExternalInput")
    wf3_d = nc.dram_tensor("wf3", [256, 40], f32, kind="ExternalInput")
    t8_d = nc.dram_tensor("t8", [1, 40], f32, kind="ExternalInput")
    gD = nc.dram_tensor("gD", [1, N], f32)
    h1D = nc.dram_tensor("h1D", [1, 512], f32)
    h2D = nc.dram_tensor("h2D", [1, 256], f32)
    out_d = nc.dram_tensor("out", [1, 40], f32, kind="ExternalOutput")

    with tile.TileContext(nc) as tc, ExitStack() as ctx:
        persist = ctx.enter_context(tc.tile_pool(name="persist", bufs=1))
        work = ctx.enter_context(tc.tile_pool(name="work", bufs=2))

        # persistent state
        xT0 = persist.tile([3, N], f32)
        nc.sync.dma_start(xT0[:], xT_d[:])
        x1T = persist.tile([64, N], f32)
        x2T = persist.tile([64, N], f32)
        x3T = persist.tile([P, N], f32)
        x4Ta = persist.tile([P, N], f32)
        x4Tb = persist.tile([P, N], f32)
        A_hi = persist.tile([P, N], f32)   # per-layer 2*xT
        A_lo = persist.tile([2, N], f32)   # [ones; d2]
        B_lo = persist.tile([2, N], f32)   # [-d2; -ones]
        sq = persist.tile([P, N], f32)
        d2row = persist.tile([1, N], f32)
        neg1row = persist.tile([1, N], f32)
        ones_col = persist.tile([P, 1], f32)
        ident = persist.tile([P, P], f32)
        nc.gpsimd.memset(ones_col[:], 1.0)
        nc.gpsimd.memset(A_lo[0:1, :], 1.0)
        nc.gpsimd.memset(neg1row[:], -1.0)
        nc.sync.dma_start(B_lo[1:2, :], neg1row[:])
        make_identity(nc, ident[:])

        # weights in SBUF
        pw_sb, aw_sb = [], []
        for li, (C, O) in enumerate(LAYERS):
            t = persist.tile([C, O], f32, name=f"pwsb{li}")
            nc.sync.dma_start(t[:], pw_d[li][:])
            pw_sb.append(t)
            t = persist.tile([C, O], f32, name=f"awsb{li}")
            nc.sync.dma_start(t[:], aw_d[li][0][:])
            tt = persist.tile([1, O], f32, name=f"awtsb{li}")
            nc.sync.dma_start(tt[:], aw_d[li][1][:])
            aw_sb.append((t, tt))
        # ---------------- layer phase (own pools, closed before head) -------
        les = ExitStack()
        a_pool = les.enter_context(tc.tile_pool(name="a_all", bufs=1))
        pd_pool = les.enter_context(tc.tile_pool(name="pd", bufs=2))
        gath_pool = les.enter_context(tc.tile_pool(name="gath", bufs=2))
        psA = les.enter_context(tc.tile_pool(name="psA", bufs=3, space="PSUM"))

        # layer input: (tensor holding transposed features, row offset)
        layer_in = [(xT0, 0), (x1T, 0), (x2T, 0), (x3T, 0)]

        for li, (C, O) in enumerate(LAYERS):
            src, ro = layer_in[li]
            inT = src[ro:ro + C, :]
            # ---- prep: d2 row, A/B augmented rows
            nc.scalar.activation(sq[0:C, :], inT[:, :], AF.Square)
            for c4 in range(4):
                d2ps = psA.tile([1, 512], f32, tag="ps", name=f"d2ps{li}_{c4}")
                nc.tensor.matmul(d2ps[:], ones_col[0:C, :], sq[0:C, bass.ts(c4, 512)],
                                 start=True, stop=True)
                nc.scalar.activation(d2row[:, bass.ts(c4, 512)], d2ps[:], AF.Copy)
            nc.sync.dma_start(A_lo[1:2, :], d2row[:])
            nc.scalar.activation(B_lo[0:1, :], d2row[:], AF.Copy, scale=-1.0)
            nc.scalar.activation(A_hi[0:C, :], inT[:, :], AF.Copy, scale=2.0)

            # ---- proj (to DRAM) and a_s (kept in SBUF) for all blocks
            a_all = a_pool.tile([P, NBLK * O], f32, tag="a_all", name=f"a_all{li}")
            for b in range(NBLK):
                bs = bass.ts(b, P)
                pp = psA.tile([P, O], f32, tag="ps", name=f"pp{li}_{b}")
                nc.tensor.matmul(pp[:], inT[:, bs], pw_sb[li][:], start=True, stop=True)
                prj = work.tile([P, O], f32, tag="prj", name=f"prj{li}_{b}")
                nc.scalar.activation(prj[:], pp[:], AF.Copy)
                nc.sync.dma_start(projD[li][bs, :], prj[:])

                pa = psA.tile([P, O], f32, tag="ps", name=f"pa{li}_{b}")
                nc.tensor.matmul(pa[:], A_hi[0:C, bs], aw_sb[li][0][:],
                                 start=True, stop=False)
                nc.tensor.matmul(pa[:], A_lo[0:1, bs], aw_sb[li][1][:],
                                 start=False, stop=True)
                nc.scalar.activation(a_all[:, bass.ts(b, O)], pa[:], AF.Copy)

            # ---- per block: pd, topk, gather, combine, transpose out
            for b in range(NBLK):
                bs = bass.ts(b, P)
                pd_sb = pd_pool.tile([P, N], f32, tag="pd", name=f"pd{li}_{b}")
                for h in range(2):
                    ph = psA.tile([P, 1024], f32, tag="pd_ps", bufs=2,
                                  name=f"ph{li}_{b}_{h}")
                    for c2 in range(2):
                        ms = bass.ds(h * 1024 + c2 * 512, 512)
                        po = ph[:, bass.ts(c2, 512)]
                        nc.tensor.matmul(po, A_hi[0:C, bs], inT[:, ms],
                                         start=True, stop=False)
                        nc.tensor.matmul(po, A_lo[:, bs], B_lo[:, ms],
                                         start=False, stop=True)
                    nc.scalar.activation(pd_sb[:, bass.ts(h, 1024)], ph[:], AF.Copy)

                idx = work.tile([P, 24], u32, tag="idx", name=f"idx{li}_{b}")
                for r in range(3):
                    vals = work.tile([P, 8], f32, tag="vals", bufs=3,
                                     name=f"vals{li}_{b}_{r}")
                    nc.vector.max(out=vals[:], in_=pd_sb[:])
                    nc.vector.max_index(out=idx[:, bass.ts(r, 8)], in_max=vals[:],
                                        in_values=pd_sb[:])
                    if r < 2:
                        nc.vector.match_replace(out=pd_sb[:], in_to_replace=vals[:],
                                                in_values=pd_sb[:], imm_value=NEG)

                gt = gath_pool.tile([P, K, O], f32, tag="gt", name=f"gt{li}_{b}")
                for k in range(K):
                    nc.gpsimd.indirect_dma_start(
                        out=gt[:, k], out_offset=None, in_=projD[li][:],
                        in_offset=IndirectOffsetOnAxis(ap=idx[:, k:k + 1], axis=0))
                m = work.tile([P, O], f32, tag="m", name=f"m{li}_{b}")
                nc.vector.tensor_reduce(m[:], gt[:].rearrange("p k o -> p o k"),
                                        axis=mybir.AxisListType.X, op=ALU.max)
                nc.vector.tensor_tensor(out=m[:], in0=m[:], in1=a_all[:, bass.ts(b, O)],
                                        op=ALU.add)
                onm = work.tile([P, O], f32, tag="onm", name=f"onm{li}_{b}")
                nc.scalar.activation(onm[:], m[:], AF.Copy, scale=0.2)
                nc.vector.tensor_tensor(out=onm[:], in0=onm[:], in1=m[:], op=ALU.max)

                # transpose out -> next layer's [O, N] layout
                if li == 0:
                    dests = [(x1T, 0)]
                elif li == 1:
                    dests = [(x2T, 0)]
                elif li == 2:
                    dests = [(x3T, 0)]
                else:
                    dests = [(x4Ta, 0), (x4Tb, -128)]
                for oc in range(0, O, P):
                    rows = min(P, O - oc)
                    pt = psA.tile([P, P], f32, tag="ps", name=f"pt{li}_{b}_{oc}")
                    nc.tensor.transpose(pt[0:rows, :], onm[:, bass.ds(oc, rows)],
                                        ident[:])
                    dst, roff = dests[oc // P]
                    nc.scalar.activation(dst[bass.ds(oc + roff, rows), bs],
                                         pt[0:rows, :], AF.Copy)

        les.close()

        # ---------------- head phase --------------------------------------
        psB = ctx.enter_context(tc.tile_pool(name="psB", bufs=2, space="PSUM"))
        hwork = ctx.enter_context(tc.tile_pool(name="hwork", bufs=2))
        hp = ctx.enter_context(tc.tile_pool(name="hpersist", bufs=1))

        maxacc = hp.tile([P, 1024], f32)
        w5_sb = []
        for c in range(5):
            rows = 64 if c < 2 else 128
            t = hp.tile([rows, 1024], f32, name=f"w5sb{c}")
            nc.sync.dma_start(t[:], w5_d[c][:])
            w5_sb.append(t)
        t5_sb = hp.tile([1, 1024], f32)
        nc.sync.dma_start(t5_sb[:], t5_d[:])
        wf2_sb = hp.tile([P, 4 * 256], f32)
        nc.sync.dma_start(wf2_sb[:].rearrange("p (c f) -> p c f", c=4), wf2_d[:].rearrange("(c p) f -> p c f", p=P))
        wf3_sb = hp.tile([P, 2 * 40], f32)
        nc.sync.dma_start(wf3_sb[:].rearrange("p (c f) -> p c f", c=2), wf3_d[:].rearrange("(c p) f -> p c f", p=P))
        t6_sb = hp.tile([1, 512], f32)
        nc.sync.dma_start(t6_sb[:], t6_d[:])
        t7_sb = hp.tile([1, 256], f32)
        nc.sync.dma_start(t7_sb[:], t7_d[:])
        t8_sb = hp.tile([1, 40], f32)
        nc.sync.dma_start(t8_sb[:], t8_d[:])

        gsum = [psB.tile([1, 512], f32, tag=f"gsum{h}", bufs=1, name=f"gsum{h}")
                for h in range(2)]
        chunks = [x1T, x2T, x3T, x4Ta, x4Tb]
        for b in range(NBLK):
            bs = bass.ts(b, P)
            for h in range(2):
                px = psB.tile([P, 512], f32, tag="ps", name=f"px{b}_{h}")
                for c, ch in enumerate(chunks):
                    nc.tensor.matmul(px[:], ch[:, bs],
                                     w5_sb[c][:, bass.ds(h * 512, 512)],
                                     start=(c == 0), stop=False)
                nc.tensor.matmul(px[:], A_lo[0:1, bs], t5_sb[:, bass.ts(h, 512)],
                                 start=False, stop=True)
                x5h = hwork.tile([P, 512], f32, tag="x5h", name=f"x5h{b}_{h}")
                nc.scalar.activation(x5h[:], px[:], AF.Copy, scale=0.2)
                nc.vector.tensor_tensor(out=x5h[:], in0=x5h[:], in1=px[:], op=ALU.max)
                if b == 0:
                    nc.vector.tensor_copy(maxacc[:, bass.ts(h, 512)], x5h[:])
                else:
                    nc.vector.tensor_tensor(out=maxacc[:, bass.ts(h, 512)],
                                            in0=maxacc[:, bass.ts(h, 512)],
                                            in1=x5h[:], op=ALU.max)
                nc.tensor.matmul(gsum[h][:], ones_col[:], x5h[:],
                                 start=(b == 0), stop=(b == NBLK - 1))

        # partition reduction: DMA-fold 128 rows -> [32, 4x1024], reduce q,
        # then 32x32 transpose + strided reduce (no cross-base DVE operands)
        t32s = hp.tile([32, 4, 1024], f32)
        for q in range(4):
            nc.sync.dma_start(t32s[:, q, :], maxacc[bass.ds(32 * q, 32), :])
        t32 = hp.tile([32, 1024], f32)
        nc.vector.tensor_reduce(t32[:], t32s[:].rearrange("p q f -> p f q"),
                                axis=mybir.AxisListType.X, op=ALU.max)
        t32b = hp.tile([32, 1024], f32)
        nc.vector.transpose(t32b[:], t32[:])
        t32 = t32b
        colmax = hp.tile([32, 32], f32)
        nc.vector.tensor_reduce(colmax[:], t32[:].rearrange("p (b j) -> p b j", j=32),
                                axis=mybir.AxisListType.X, op=ALU.max)
        # colmax[i, b] holds max of x5 column 32*b + i
        nc.sync.dma_start(gD[:, 0:1024].rearrange("a (b i) -> (a i) b", i=32),
                          colmax[:])
        gsb = hp.tile([1, N], f32)
        for h in range(2):
            nc.scalar.activation(gsb[:, bass.ds(1024 + h * 512, 512)], gsum[h][:],
                                 AF.Copy, scale=1.0 / N)
        nc.sync.dma_start(gD[:, 1024:2048], gsb[:, 1024:2048])
        gcol = hp.tile([P, 16], f32)
        nc.sync.dma_start(gcol[:], gD[:].rearrange("a (c p) -> (a p) c", p=P))

        # ---- fc1: [1,2048]@[2048,512]
        f1ps = psB.tile([1, 512], f32, tag="f1ps", bufs=1)
        for c in range(16):
            wchunk = hwork.tile([P, 512], f32, tag="wf1c", name=f"wf1c{c}")
            nc.sync.dma_start(wchunk[:], wf1_d[bass.ts(c, P), :])
            nc.tensor.matmul(f1ps[:], gcol[:, c:c + 1], wchunk[:],
                             start=(c == 0), stop=(c == 15))
        h1 = hp.tile([1, 512], f32)
        nc.vector.tensor_tensor(out=h1[:], in0=f1ps[:], in1=t6_sb[:], op=ALU.add)
        h1b = hp.tile([1, 512], f32)
        nc.scalar.activation(h1b[:], h1[:], AF.Copy, scale=0.2)
        nc.vector.tensor_tensor(out=h1[:], in0=h1[:], in1=h1b[:], op=ALU.max)
        nc.sync.dma_start(h1D[:], h1[:])
        h1col = hp.tile([P, 4], f32)
        nc.sync.dma_start(h1col[:], h1D[:].rearrange("a (c p) -> (a p) c", p=P))

        # ---- fc2: [1,512]@[512,256]
        f2ps = psB.tile([1, 256], f32, tag="f2ps", bufs=1)
        for c in range(4):
            nc.tensor.matmul(f2ps[:], h1col[:, c:c + 1], wf2_sb[:, bass.ts(c, 256)],
                             start=(c == 0), stop=(c == 3))
        h2 = hp.tile([1, 256], f32)
        nc.vector.tensor_tensor(out=h2[:], in0=f2ps[:], in1=t7_sb[:], op=ALU.add)
        h2b = hp.tile([1, 256], f32)
        nc.scalar.activation(h2b[:], h2[:], AF.Copy, scale=0.2)
        nc.vector.tensor_tensor(out=h2[:], in0=h2[:], in1=h2b[:], op=ALU.max)
        nc.sync.dma_start(h2D[:], h2[:])
        h2col = hp.tile([P, 2], f32)
        nc.sync.dma_start(h2col[:], h2D[:].rearrange("a (c p) -> (a p) c", p=P))

        # ---- fc3: [1,256]@[256,40]
        f3ps = psB.tile([1, 40], f32, tag="f3ps", bufs=1)
        for c in range(2):
            nc.tensor.matmul(f3ps[:], h2col[:, c:c + 1], wf3_sb[:, bass.ts(c, 40)],
                             start=(c == 0), stop=(c == 1))
        ofin = hp.tile([1, 40], f32)
        nc.vector.tensor_tensor(out=ofin[:], in0=f3ps[:], in1=t8_sb[:], op=ALU.add)
        nc.sync.dma_start(out_d[:], ofin[:])

    nc.compile()
    return nc


# ------------------------------------------------------------- host glue ---
def _fold_params(I):
    def conv(w, bn):
        O, twoC = w.shape
        C = twoC // 2
        g, b, m, v = bn
        s = g / np.sqrt(v + EPS)
        t = b - m * s
        wd, wc = w[:, :C], w[:, C:]
        Pw = (wd * s[:, None]).T.astype(np.float32)
        Aw = ((wc - wd) * s[:, None]).T.astype(np.float32)
        return (np.ascontiguousarray(Pw),
                np.ascontiguousarray((Aw / 2.0).astype(np.float32)),
                np.ascontiguousarray(t[None, :].astype(np.float32)))

    out = {}
    for li, wk, bk in [(0, "w1", "bn1"), (1, "w2", "bn2"), (2, "w3", "bn3"),
                       (3, "w4", "bn4")]:
        pw, aw, awt = conv(I[wk], I[bk])
        out[f"pw{li}"] = pw
        out[f"aw{li}"] = aw
        out[f"awt{li}"] = awt

    def fc(w, bn):
        g, b, m, v = bn
        s = g / np.sqrt(v + EPS)
        t = b - m * s
        return (np.ascontiguousarray((w * s[:, None]).T.astype(np.float32)),
                t.astype(np.float32))

    w5m, t5 = fc(I["w5"], I["bn5"])
    bnds = [0, 64, 128, 256, 384, 512]
    for c in range(5):
        out[f"w5c{c}"] = np.ascontiguousarray(w5m[bnds[c]:bnds[c + 1]])
    out["t5"] = t5[None, :]
    wf1, t6 = fc(I["wl1"], I["bn6"])
    out["wf1"], out["t6"] = wf1, t6[None, :]
    g7, b7, m7, v7 = I["bn7"]
    s7 = g7 / np.sqrt(v7 + EPS)
    t7 = b7 - m7 * s7
    out["wf2"] = np.ascontiguousarray((I["wl2"] * s7[:, None]).T.astype(np.float32))
    out["t7"] = (I["bl2"] * s7 + t7).astype(np.float32)[None, :]
    out["wf3"] = np.ascontiguousarray(I["wl3"].T.astype(np.float32))
    out["t8"] = I["bl3"].astype(np.float32)[None, :]
    return out


_NC_CACHE = None


def get_nc():
    global _NC_CACHE
    if _NC_CACHE is None:
        _NC_CACHE = build_program()
    return _NC_CACHE


def make_in_maps(inputs):
    I = {k: np.asarray(v) for k, v in inputs.items()}
    params = _fold_params(I)
    B = I["x"].shape[0]
    in_maps = []
    for b in range(B):
        m = dict(params)
        m["xT"] = np.ascontiguousarray(I["x"][b].T.astype(np.float32))
        in_maps.append(m)
    return in_maps


def kernel(**inputs):
    nc = get_nc()
    in_maps = make_in_maps(inputs)
    res = run_bass_kernel_spmd(nc, in_maps, list(range(len(in_maps))))
    return np.stack([r["out"][0] for r in res.results]).astype(np.float32)

